# revision 1
# baseline (speedup 1.0000x reference)
"""Trainium2 Bass kernel for nn_Decoder (LSTM decoder + fc1/relu/fc2 head).

Strategy (8 NeuronCores, data-parallel over batch, 32 rows/core):
  - The 511-step LSTM recurrence runs fully in TRANSPOSED space: the
    state hT/cT live as [128 h-dims, 32 batch] column groups, the gate
    matmuls put the gate dimension on PSUM partitions (lhsT = static
    w_hh blocks, rhs = hT state slices), so no per-step transposes are
    ever needed. x*w_ih + bias enter via a K=2 matmul whose lhsT is a
    static [2,128] block ([w_ih; bias]) and rhs is a precomputed
    [trg_t; ones] column pair. The g-gate rows are pre-scaled by 2 on
    the host so ONE sigmoid activation evaluates all four gates
    (tanh(x) = 2*sigmoid(2x) - 1).
  - Matmuls use float32r (full-rate fp32 path on the PE).
  - Head: fc1+relu computed transposed the same way -> zT, then fc2
    streams the 131MB fc2_w.T from HBM in [128, 2000] tiles.
  - Host side: shard batch, pre-permute weights, concat core outputs.

Runtime note: on this runtime, cross-engine dependency hops and
ACT-engine ops are orders of magnitude more expensive than the cost
model predicts, so the design minimizes instruction count on ACT/DVE
and semaphore hops per step rather than PE streaming cycles.
"""

import sys

sys.path.insert(0, "/opt/trn_rl_repo")

import ml_dtypes
import numpy as np
from contextlib import ExitStack

import concourse.bass as bass
import concourse.mybir as mybir
import concourse.tile as tile
from concourse.bass_utils import run_bass_kernel_spmd

F32 = mybir.dt.float32
F32R = mybir.dt.float32r
BF16 = mybir.dt.bfloat16
F16 = mybir.dt.float16
FP8E4 = mybir.dt.float8e4
AFT = mybir.ActivationFunctionType
ALU = mybir.AluOpType

N_CORES = 8
B = 256
BSH = B // N_CORES  # 32 batch rows per core
H = 512
G = 4 * H  # 2048
HID = 1024
V = 32000
T_STEPS = 511  # LSTM consumes trg[:, 0:511]

NW = 2000   # fc2 vocab window
NBANK = 500  # fc2 bank width (4 banks per window, 512-aligned in psum)
N_WIN = V // NW  # 16

_MAX_WAITS = 1


def _split_multi_waits(nc):
    """This walrus accepts at most one sync-wait per TPB instruction.
    Move extra waits onto same-engine nops placed directly before the
    instruction (engines execute their stream in order)."""
    ctr = 0
    for fn in nc.m.functions:
        for bb in fn.blocks:
            insts = list(bb.instructions)
            out = []
            changed = False
            for inst in insts:
                si = inst.sync_info
                if si is not None and si.on_wait and len(si.on_wait) > _MAX_WAITS:
                    waits = list(si.on_wait)
                    for w in waits[:-_MAX_WAITS]:
                        ctr += 1
                        nop = mybir.InstNoOp(
                            name=f"swsplit-{ctr}",
                            engine=inst.engine,
                            bass_nofuse=True,
                            sync_info=mybir.SyncInfo(on_wait=[w], on_update=[]),
                        )
                        nc.register_instruction(nop, overwrite=True)
                        out.append(nop)
                    si.on_wait = waits[-_MAX_WAITS:]
                    changed = True
                out.append(inst)
            if changed:
                bb.instructions = out


def _thin_pe_sem_updates(nc):
    """Per-MM semaphore increments cost ~26ns each of serialized EVT_SEM
    writes on the PE. Consumers only wait on a handful of counts per loop
    iteration, so drop the updates nobody waits on and renumber the rest.
    Exact-producer preserving: every wait still waits on the same
    instruction. Aborts (no-op) on any unexpected structure."""
    for fn in nc.m.functions:
        blocks = list(fn.blocks)
        bodies = [bb for bb in blocks
                  if "-lstm" in bb.name and bb.name.endswith("_body")]
        if not bodies:
            continue
        pe_sem = None
        for inst in bodies[0].instructions:
            si = inst.sync_info
            if isinstance(inst, mybir.InstMatmult) and si and si.on_update:
                for u in si.on_update:
                    if u.update_mode == "sem-inc" and u.update_value == 1:
                        pe_sem = u.ant_name
                if pe_sem:
                    break
        if pe_sem is None:
            return
        # ordered +1 updates per body; all bodies must look identical
        body_upds = []
        for bb in bodies:
            upds = []
            for inst in bb.instructions:
                si = inst.sync_info
                if si and si.on_update:
                    for u in si.on_update:
                        if u.ant_name == pe_sem:
                            if not (u.update_mode == "sem-inc"
                                    and u.update_value == 1):
                                return  # unexpected
                            upds.append((inst, u))
            body_upds.append(upds)
        N = len(body_upds[0])
        if N == 0 or any(len(u) != N for u in body_upds):
            return
        n_inst = len(bodies)
        # c0 from instance 0's skip block (waits base, adds N)
        c0 = None
        for bb in blocks:
            if "-lstm" in bb.name and bb.name.endswith("_skip"):
                for inst in bb.instructions:
                    si = inst.sync_info
                    if (si and si.on_wait and si.on_update
                            and any(u.ant_name == pe_sem and
                                    u.update_mode == "sem-add-imm"
                                    for u in si.on_update)):
                        for w in si.on_wait:
                            if w.ant_name == pe_sem:
                                c0 = w.wait_value
                                break
                break
        if c0 is None:
            return
        # pass 1: collect all waits on pe_sem; compute kept offsets
        all_waits = []
        for bb in blocks:
            for inst in bb.instructions:
                si = inst.sync_info
                if si and si.on_wait:
                    for w in si.on_wait:
                        if w.ant_name == pe_sem:
                            if w.wait_mode != "sem-ge-imm":
                                return
                            all_waits.append(w)
        kept = {N}
        for w in all_waits:
            v = w.wait_value
            if v <= c0:
                continue
            r = (v - c0 - 1) // N
            if r < n_inst:
                kept.add(v - c0 - r * N)
        kept_sorted = sorted(kept)
        K = len(kept_sorted)
        rank = {off: i + 1 for i, off in enumerate(kept_sorted)}
        # validate reset/skip totals BEFORE mutating anything
        totals = []
        for bb in blocks:
            if "-lstm" in bb.name and (bb.name.endswith("_reset")
                                       or bb.name.endswith("_skip")):
                for inst in bb.instructions:
                    si = inst.sync_info
                    if si and si.on_update:
                        for u in si.on_update:
                            if (u.ant_name == pe_sem and u.update_mode in
                                    ("sem-sub-imm", "sem-add-imm")):
                                totals.append(u)
        if any(u.update_value != N for u in totals):
            return
        # pass 2a: rewrite waits
        for w in all_waits:
            v = w.wait_value
            if v <= c0:
                continue
            r = (v - c0 - 1) // N
            if r < n_inst:
                off = v - c0 - r * N
                w.wait_value = c0 + r * K + rank[off]
            else:
                w.wait_value = v - n_inst * (N - K)
        # pass 2b: drop unkept +1 updates in bodies
        for upds in body_upds:
            for i, (inst, u) in enumerate(upds):
                if (i + 1) not in kept:
                    inst.sync_info.on_update = [
                        x for x in inst.sync_info.on_update if x is not u]
        # pass 2c: fix reset sub-imm / skip add-imm totals
        for u in totals:
            u.update_value = K


class _SplitDrainTileContext(tile.TileContext):
    def schedule_and_allocate(self):
        ret = super().schedule_and_allocate()
        if _THIN_PE_SEMS:
            _thin_pe_sem_updates(self.nc)
        _split_multi_waits(self.nc)
        return ret


_THIN_PE_SEMS = True


def _build_program(n_steps=T_STEPS, n_loops=1, body=3, unroll=2, wdt=F16,
                   w8=False):
    # body bitmask (timing): 1=sigma+chain, 2=all-16-m (else 4)
    nc = bass.Bass("TRN2", target_bir_lowering=False, debug=False, num_devices=1)
    ns = max(n_steps, 1)
    assert n_steps == 1 or (n_steps - 1) % unroll == 0

    # wt2[p, (m*5+k)*128 + j] = w_hh_scaled[mrow(m,j), k*128+p]  (lhsT blocks;
    # k=4 is the input/bias injection block). With w8, the w_hh blocks live in
    # a separate fp8e4m3 tensor and the injection blocks stay 16-bit in wt5.
    nblk = 64 if w8 else 80
    wt2_d = nc.dram_tensor("wt2", [128, nblk * 128], FP8E4 if w8 else wdt,
                           kind="ExternalInput").ap()
    wt5_d = (nc.dram_tensor("wt5", [128, 16 * 128], wdt,
                            kind="ExternalInput").ap() if w8 else None)
    trga_d = nc.dram_tensor("trga", [2, ns * BSH], wdt, kind="ExternalInput").ap()
    fc1t2_d = nc.dram_tensor("fc1t2", [128, 32 * 128], wdt,
                             kind="ExternalInput").ap()
    fc1b2_d = nc.dram_tensor("fc1b2", [1, HID], wdt, kind="ExternalInput").ap()
    fc2t_d = nc.dram_tensor("fc2t", [8, 128, V], BF16, kind="ExternalInput").ap()
    fc2b_d = nc.dram_tensor("fc2b", [1, V], BF16, kind="ExternalInput").ap()
    onesb_d = nc.dram_tensor("onesb", [1, BSH], BF16, kind="ExternalInput").ap()
    ones_d = nc.dram_tensor("onesr", [1, BSH], wdt, kind="ExternalInput").ap()
    zi_d = nc.dram_tensor("zi", [128, 128], wdt, kind="ExternalInput").ap()
    out_d = nc.dram_tensor("out", [BSH, V], F32, kind="ExternalOutput").ap()

    with _SplitDrainTileContext(nc) as tc, ExitStack() as ctx:
        const = ctx.enter_context(tc.tile_pool(name="const", bufs=1))
        state = ctx.enter_context(tc.tile_pool(name="state", bufs=1))
        work = ctx.enter_context(tc.tile_pool(name="work", bufs=1))

        wt2 = const.tile([128, nblk * 128], FP8E4 if w8 else wdt)
        nc.sync.dma_start(wt2[:], wt2_d[:])
        if w8:
            wt5 = const.tile([128, 16 * 128], wdt)
            nc.sync.dma_start(wt5[:], wt5_d[:])
        trgaux = const.tile([2, ns * BSH], wdt)
        nc.sync.dma_start(trgaux[:], trga_d[:])
        fc1t2 = const.tile([128, 32 * 128], wdt)
        nc.sync.dma_start(fc1t2[:], fc1t2_d[:])
        fc1b2 = const.tile([1, HID], wdt)
        nc.sync.dma_start(fc1b2[:], fc1b2_d[:])
        ones = const.tile([1, BSH], wdt)
        nc.sync.dma_start(ones[:], ones_d[:])
        ones_bf = const.tile([1, BSH], BF16)
        nc.sync.dma_start(ones_bf[:], onesb_d[:])

        # state, transposed space: col group b = h-chunk b ([128] x [32])
        cT = state.tile([128, 128], F16)
        hT = state.tile([128, 128], wdt)
        nc.vector.memset(cT[:], 0.0)
        nc.sync.dma_start(hT[:], zi_d[:])

        acts = work.tile([128, 512], F16)  # act(gatesT): [tg|i|f|o] x4
        tg = work.tile([128, 128], F16)
        t1 = work.tile([128, 128], F16)
        tc_ = work.tile([128, 128], F16)

        # xa[0:2, slot*BSH+b] = [x_t; 1]; rows 2..127 stay zero so the
        # input/bias injection is a plain 5th K-chunk (FWL-rate LDW).
        xa = work.tile([128, (unroll + 1) * BSH], wdt)
        nc.vector.memset(xa[:], 0.0)

        def emit_step(xslot, pgA, pgB):
            # gates g,i accumulate in pgA; f,o in pgB (separate PSUM banks
            # so the half-1 activations never WAR-serialize half-2 matmuls)
            xsl = xa[:, xslot * BSH:(xslot + 1) * BSH]
            nm = 16 if body & 2 else 4
            kb = 4 if w8 else 5
            for m in range(nm):
                pg = pgA if m < 8 else pgB
                outm = pg[:, (m % 8) * 32:(m % 8 + 1) * 32]
                # inject first (start=True): it has no h dependency, so the
                # PE can run it during the previous step's chain.
                inj = (wt5[:, m * 128:(m + 1) * 128] if w8 else
                       wt2[:, (m * 5 + 4) * 128:(m * 5 + 5) * 128])
                nc.tensor.matmul(outm, lhsT=inj, rhs=xsl,
                                 start=True, stop=False)
                for k in range(4):
                    nc.tensor.matmul(
                        outm,
                        lhsT=wt2[:, (m * kb + k) * 128:(m * kb + k + 1) * 128],
                        rhs=hT[:, k * 32:(k + 1) * 32],
                        start=False, stop=(k == 3))
                if body & 1 and m == nm // 2 - 1:
                    # first half done: gates g (cols 0:128) and i (128:256)
                    nc.scalar.activation(tg[:], pgA[:, 0:128], AFT.Tanh)
                    nc.scalar.activation(acts[:, 128:256], pgA[:, 128:256],
                                         AFT.Sigmoid)
                    nc.vector.tensor_mul(t1[:], acts[:, 128:256], tg[:])

            if not body & 1:
                return
            # second half: f (256:384), o (384:512)
            nc.scalar.activation(acts[:, 256:512], pgB[:, 0:256], AFT.Sigmoid)
            nc.vector.tensor_mul(cT[:], cT[:], acts[:, 256:384])
            nc.vector.tensor_add(cT[:], cT[:], t1[:])
            nc.scalar.activation(tc_[:], cT[:], AFT.Tanh)
            nc.vector.tensor_mul(hT[:], acts[:, 384:512], tc_[:])

        with tc.tile_pool(name="psum_g", bufs=1, space="PSUM") as pg_pool:
            pgA0 = pg_pool.tile([128, 256], F32, tag="pa0", name="pgA0")
            pgB0 = pg_pool.tile([128, 256], F32, tag="pb0", name="pgB0")
            pgA1 = pg_pool.tile([128, 256], F32, tag="pa1", name="pgA1")
            pgB1 = pg_pool.tile([128, 256], F32, tag="pb1", name="pgB1")
            # t = 0 prologue (static), then (n_steps-1)/unroll-iter hw loop
            nc.vector.tensor_copy(xa[0:2, 0:BSH], trgaux[:, 0:BSH])
            emit_step(0, pgA0, pgB0)
            pgs = [(pgA1, pgB1), (pgA0, pgB0)]
            for _rep in range(n_loops if n_steps > 1 else 0):
                assert n_steps == 511
                with tc.For_i(1, n_steps, unroll, name=f"lstm{_rep}") as tv:
                    off = tv * BSH
                    nc.vector.tensor_copy(
                        xa[0:2, BSH:(unroll + 1) * BSH],
                        trgaux[:, bass.ds(off, unroll * BSH)])
                    for u in range(unroll):
                        emit_step(1 + u, *pgs[u % 2])

        # ---- head: fc1 transposed (zT directly), then fc2 ----
        zT = work.tile([128, 256], BF16)
        with tc.tile_pool(name="psum_z", bufs=1, space="PSUM") as pz_pool:
            pzT = pz_pool.tile([128, 256], F32)  # 8 m-chunks x 32
            for m in range(8):
                outm = pzT[:, m * 32:(m + 1) * 32]
                for k in range(4):
                    nc.tensor.matmul(
                        outm,
                        lhsT=fc1t2[:, (m * 4 + k) * 128:(m * 4 + k + 1) * 128],
                        rhs=hT[:, k * 32:(k + 1) * 32],
                        start=(k == 0), stop=False)
                nc.tensor.matmul(
                    outm, lhsT=fc1b2[:, m * 128:(m + 1) * 128],
                    rhs=ones[:], start=False, stop=True)
            nc.scalar.activation(zT[:], pzT[:], AFT.Relu)

        with tc.tile_pool(name="fcw", bufs=3) as fcw_pool, \
             tc.tile_pool(name="fbw", bufs=2) as fbw_pool, \
             tc.tile_pool(name="outw", bufs=2) as out_pool, \
             tc.tile_pool(name="psum_w", bufs=2, space="PSUM") as pw_pool:
            for w in range(N_WIN):
                w0 = w * NW
                pw = pw_pool.tile([BSH, 4 * 512], F32)
                fbt = fbw_pool.tile([1, NW], BF16)
                nc.sync.dma_start(fbt[:], fc2b_d[:, w0:w0 + NW])
                for kc in range(8):
                    wt_f = fcw_pool.tile([128, NW], BF16, tag="fcw")
                    nc.sync.dma_start(wt_f[:], fc2t_d[kc, :, w0:w0 + NW])
                    for nb in range(4):
                        nc.tensor.matmul(
                            pw[:, nb * 512: nb * 512 + NBANK],
                            lhsT=zT[:, kc * 32:(kc + 1) * 32],
                            rhs=wt_f[:, nb * NBANK:(nb + 1) * NBANK],
                            start=(kc == 0), stop=False,
                            skip_group_check=True)
                for nb in range(4):
                    nc.tensor.matmul(
                        pw[:, nb * 512: nb * 512 + NBANK],
                        lhsT=ones_bf[:],
                        rhs=fbt[:, nb * NBANK:(nb + 1) * NBANK],
                        start=False, stop=True, skip_group_check=True)
                ot = out_pool.tile([BSH, NW], F32)
                for nb in range(4):
                    nc.scalar.activation(
                        ot[:, nb * NBANK:(nb + 1) * NBANK],
                        pw[:, nb * 512: nb * 512 + NBANK], AFT.Copy)
                nc.sync.dma_start(out_d[:, w0:w0 + NW], ot[:])

    return nc


def _prep_host(x, hidden, trg, w_ih, w_hh, b_ih, b_hh, fc1_w, fc1_b, fc2_w,
               fc2_b, n_steps=T_STEPS, wdt_np=np.float16, w8=False):
    """Host-side weight permutation + per-core input maps."""
    f32 = np.float32
    ns = max(n_steps, 1)
    w_hh = np.asarray(w_hh, f32)
    w_ih = np.asarray(w_ih, f32).reshape(-1)
    bias = (np.asarray(b_ih, f32) + np.asarray(b_hh, f32)).reshape(-1)

    # m-chunk order: [g0..3, i0..3, f0..3, o0..3]; torch row blocks i,f,g,o
    blkmap = np.array([2, 0, 1, 3])  # g,i,f,o -> torch block index
    mrows = np.concatenate([
        blkmap[gt] * 512 + hc * 128 + np.arange(128)
        for gt in range(4) for hc in range(4)])          # [2048] W row ids
    wsc = w_hh[mrows]                                    # [2048, 512]
    wihs = w_ih[mrows]                                   # [2048]
    biass = bias[mrows]                                  # [2048]
    # wt2[p, (m*kb+k)*128 + j]: k<4 -> w_hh lhsT blocks; k=4 (or wt5 when
    # w8) -> input/bias injection block (row 0 = w_ih, row 1 = bias).
    kb = 4 if w8 else 5
    wt2 = np.zeros((128, 16 * kb * 128), f32)
    wt5 = np.zeros((128, 16 * 128), f32)
    for m in range(16):
        for k in range(4):
            blk = wsc[m * 128:(m + 1) * 128, k * 128:(k + 1) * 128]
            wt2[:, (m * kb + k) * 128:(m * kb + k + 1) * 128] = blk.T
        w5sl = (wt5[:, m * 128:(m + 1) * 128] if w8 else
                wt2[:, (m * 5 + 4) * 128:(m * 5 + 5) * 128])
        w5sl[0] = wihs[m * 128:(m + 1) * 128]
        w5sl[1] = biass[m * 128:(m + 1) * 128]
    if w8:
        wt2 = wt2.astype(ml_dtypes.float8_e4m3)
    else:
        wt2 = wt2.astype(wdt_np)
    wt5 = wt5.astype(wdt_np)

    fc1_w = np.asarray(fc1_w, f32)
    f4 = fc1_w.reshape(8, 128, 4, 128)                   # [m, j, k, p]
    fc1t2 = np.ascontiguousarray(
        np.transpose(f4, (3, 0, 2, 1)).reshape(128, 32 * 128)).astype(wdt_np)
    fc1b2 = np.asarray(fc1_b, f32).reshape(1, HID).astype(wdt_np)

    bf16 = ml_dtypes.bfloat16
    fc2t = np.ascontiguousarray(
        np.asarray(fc2_w, f32).T.reshape(8, 128, V).astype(bf16))
    fc2bv = np.asarray(fc2_b, f32).reshape(1, V).astype(bf16)

    trg_f = np.asarray(trg)[:, :n_steps].astype(f32)     # [B, n_steps]
    in_maps = []
    for c in range(N_CORES):
        sh = trg_f[c * BSH:(c + 1) * BSH]                # [BSH, n_steps]
        trga = np.ones((2, ns * BSH), f32)
        trga[0, :n_steps * BSH] = sh.T.reshape(-1)
        imap = {
            "wt2": wt2, "trga": trga.astype(wdt_np),
            "fc1t2": fc1t2, "fc1b2": fc1b2,
            "fc2t": fc2t, "fc2b": fc2bv,
            "onesr": np.ones((1, BSH), wdt_np),
            "onesb": np.ones((1, BSH), bf16),
            "zi": np.zeros((128, 128), wdt_np),
        }
        if w8:
            imap["wt5"] = wt5
        in_maps.append(imap)
    return in_maps


_CACHE = {}

# production configuration (build kwargs / host-prep kwargs)
PROD_CFG = {"build": {}, "prep": {}}


def _get_program(n_steps=T_STEPS):
    if n_steps not in _CACHE:
        _CACHE[n_steps] = _build_program(n_steps, **PROD_CFG["build"])
    return _CACHE[n_steps]


def kernel(**inputs):
    nc = _get_program(T_STEPS)
    in_maps = _prep_host(**{k: inputs[k] for k in (
        "x", "hidden", "trg", "w_ih", "w_hh", "b_ih", "b_hh",
        "fc1_w", "fc1_b", "fc2_w", "fc2_b")}, n_steps=T_STEPS,
        **PROD_CFG["prep"])
    res = run_bass_kernel_spmd(nc, in_maps, core_ids=list(range(N_CORES)))
    out = np.concatenate([res.results[c]["out"] for c in range(N_CORES)], axis=0)
    return out.astype(np.float32)

